# revision 5
# baseline (speedup 1.0000x reference)
"""Trainium2 Bass kernel for nn_BasicTransformerBlock_12738873000028.

Strategy (8 NeuronCores): data-parallel over batch (2) x sequence-parallel over
query rows (4) => core c handles batch c//4, query rows [(c%4)*1024, +1024).

Everything on-chip runs in "transposed" layout (channels on SBUF partitions,
tokens on the free dim), so every matmul contracts over the partition dim with
zero on-chip transposes. Host-side numpy does the layout transposes, bf16
casts, bias-row augmentation and sharding; matmuls are bf16 with fp32 PSUM
accumulation, everything else (softmax, norm stats, residuals) is fp32.

Softmax denominator comes from a ones-column appended to V (one extra PSUM
row); biases ride as an extra contraction row (ones row in the activations,
bias row in the weights). The group-norm statistics are the only cross-core
dependency: a 16x2 fp32 AllReduce within each batch's 4-core group.
"""

import numpy as np
import ml_dtypes

import concourse.bacc as bacc
import concourse.tile as tile
from concourse import mybir
from concourse.bass_utils import run_bass_kernel_spmd

bf16 = ml_dtypes.bfloat16
F32 = mybir.dt.float32
BF16 = mybir.dt.bfloat16

B, H, W, C = 2, 64, 64, 320
N = H * W                      # 4096 tokens per batch
NCORES = 8
QL = N // 4                    # 1024 local query rows per core
QWIN = 512                     # query window (fp32 PSUM bank = 512 floats)
NQW = QL // QWIN               # 2 windows
C8 = 8 * C                     # 2560
C4 = 4 * C                     # 1280
GROUPS, EPS = 16, 1e-3
GSIZE = C // GROUPS            # 20 channels per group
GCNT = float(N * GSIZE)        # elements per (batch, group)
MT = N // 128                  # 32 key tiles
HT8 = C8 // 128                # 20 geglu output tiles
HT4 = C4 // 128                # 10 per half
SCALE = float(C) ** -0.5

# channel tiling: (offset, size, augmented-size)
CT = [(0, 128, 128), (128, 128, 128), (256, 64, 65)]

_NC_CACHE = {}


def _emit_body(nc, tc, ap, pools):
    """Emit one full forward pass. ap: dict of DRAM APs. pools: tile pools."""
    res, ps_acc, ps_mm, ps_tiny, dram = (
        pools["res"], pools["acc"], pools["mm"], pools["tiny"], pools["dram"])

    def rtile(shape, dtype, tag):
        return res.tile(shape, dtype, tag=tag, name=tag)

    # ---------------- resident loads ----------------
    X16 = []   # x^T bf16 (+ones row), full batch [321, 4096]
    XQ16 = []  # x^T bf16 local query cols (+ones row) [321, 1024]
    XF = []    # x^T fp32 local [320, 1024]
    GB = []    # gamma/beta [320, 2]
    INDsb = []
    for i, (off, sz, asz) in enumerate(CT):
        t = rtile([asz, N], BF16, f"x16_{i}")
        nc.sync.dma_start(out=t, in_=ap["xt16"][off:off + asz, :])
        X16.append(t)
        t = rtile([asz, QL], BF16, f"xq16_{i}")
        nc.sync.dma_start(out=t, in_=ap["xq16"][off:off + asz, :])
        XQ16.append(t)
        t = rtile([sz, QL], F32, f"xf_{i}")
        nc.sync.dma_start(out=t, in_=ap["xt32"][off:off + sz, :])
        XF.append(t)
        t = rtile([sz, 2], F32, f"gb_{i}")
        nc.sync.dma_start(out=t, in_=ap["gb"][off:off + sz, :])
        GB.append(t)
        t = rtile([sz, GROUPS], F32, f"ind_{i}")
        nc.sync.dma_start(out=t, in_=ap["ind"][off:off + sz, :])
        INDsb.append(t)
    INDT = rtile([GROUPS, C], F32, "indt")
    nc.sync.dma_start(out=INDT, in_=ap["indt"][:, :])

    def load_w(name, ncols):
        tiles = []
        for i, (off, sz, asz) in enumerate(CT):
            t = rtile([asz, ncols], BF16, f"w_{name}_{i}")
            nc.sync.dma_start(out=t, in_=ap[name][off:off + asz, :])
            tiles.append(t)
        return tiles

    SAQ = load_w("saq", C)
    SAK = load_w("sak", C)
    SAV = load_w("sav", C)
    SAP = load_w("sap", C)
    CAQ = load_w("caq", C)
    CAK = load_w("cak", C)
    CAV = load_w("cav", C)
    CAP = load_w("cap", C)
    GW = load_w("gw", C8)
    DW = []
    for j in range(HT4):
        t = rtile([128, C], BF16, f"dw_{j}")
        nc.sync.dma_start(out=t, in_=ap["dw"][j * 128:(j + 1) * 128, :])
        DW.append(t)
    DB = rtile([1, C], BF16, "db")
    nc.sync.dma_start(out=DB, in_=ap["db"][:, :])

    # persistent on-chip state
    K16 = [rtile([sz, N], BF16, f"k16_{i}") for i, (_, sz, _) in enumerate(CT)]
    V16 = rtile([128, MT * 321], BF16, "v16")   # per key-tile: 320 cols V + 1 col ones
    Q16 = [rtile([sz, QL], BF16, f"q16_{i}") for i, (_, sz, _) in enumerate(CT)]
    X2 = [rtile([sz, QL], F32, f"x2_{i}") for i, (_, sz, _) in enumerate(CT)]
    X3 = [rtile([sz, QL], F32, f"x3_{i}") for i, (_, sz, _) in enumerate(CT)]
    XN16 = [rtile([asz, QL], BF16, f"xn16_{i}") for i, (_, _, asz) in enumerate(CT)]
    X316 = [rtile([asz, QL], BF16, f"x316_{i}") for i, (_, _, asz) in enumerate(CT)]
    YT = [rtile([sz, QL], F32, f"xf_{i}") for i, (_, sz, _) in enumerate(CT)]  # reuse xf slots
    ONES32 = rtile([1, 128], F32, "ones32")
    ONES16 = rtile([1, QWIN], BF16, "ones16")

    nc.vector.memset(ONES32, 1.0)
    nc.vector.memset(ONES16, 1.0)
    # ones column of every V key-tile block
    v_as_blocks = V16.rearrange("p (t c) -> p t c", c=321)
    nc.vector.memset(v_as_blocks[:, :, 320:321], 1.0)
    nc.vector.memset(XN16[2][64:65, :], 1.0)
    nc.vector.memset(X316[2][64:65, :], 1.0)

    def kv_proj(SRC16, WK, WV):
        """K^T[c, m] for all m into K16; V[m, c] (+ones col) into V16."""
        for i, (off, sz, _) in enumerate(CT):
            for mw in range(N // QWIN):
                pk = ps_mm.tile([sz, QWIN], F32, tag="mm", name="mm")
                for ci in range(3):
                    nc.tensor.matmul(
                        pk, WK[ci][:, off:off + sz],
                        SRC16[ci][:, mw * QWIN:(mw + 1) * QWIN],
                        start=(ci == 0), stop=(ci == 2))
                nc.scalar.copy(out=K16[i][:, mw * QWIN:(mw + 1) * QWIN], in_=pk)
        for mt in range(MT):
            pv = ps_mm.tile([128, C], F32, tag="mm", name="mm")
            for ci in range(3):
                nc.tensor.matmul(
                    pv, SRC16[ci][:, mt * 128:(mt + 1) * 128], WV[ci][:, :],
                    start=(ci == 0), stop=(ci == 2))
            nc.vector.tensor_copy(out=V16[:, mt * 321:mt * 321 + C], in_=pv)

    def q_proj(SRC16, WQ):
        for i, (off, sz, _) in enumerate(CT):
            for qw in range(NQW):
                pq = ps_mm.tile([sz, QWIN], F32, tag="mm", name="mm")
                for ci in range(3):
                    nc.tensor.matmul(
                        pq, WQ[ci][:, off:off + sz],
                        SRC16[ci][:, qw * QWIN:(qw + 1) * QWIN],
                        start=(ci == 0), stop=(ci == 2))
                nc.scalar.copy(out=Q16[i][:, qw * QWIN:(qw + 1) * QWIN], in_=pq)

    def attention_core(WP, resid_fn):
        """scores -> softmax -> SV -> div -> proj; resid_fn(co, qw, psum_p)."""
        for qw in range(NQW):
            qsl = slice(qw * QWIN, (qw + 1) * QWIN)
            po = [ps_acc.tile([asz, QWIN], F32, tag="acc", name="acc") for (_, _, asz) in CT]
            for mt in range(MT):
                psc = ps_mm.tile([128, QWIN], F32, tag="mm", name="mm")
                for ci in range(3):
                    nc.tensor.matmul(
                        psc, K16[ci][:, mt * 128:(mt + 1) * 128], Q16[ci][:, qsl],
                        start=(ci == 0), stop=(ci == 2))
                es = res.tile([128, QWIN], BF16, tag="es", name="es", bufs=3)
                nc.scalar.activation(out=es, in_=psc, func=mybir.ActivationFunctionType.Exp)
                for cj, (off, sz, asz) in enumerate(CT):
                    nc.tensor.matmul(
                        po[cj], V16[:, mt * 321 + off:mt * 321 + off + asz], es,
                        start=(mt == 0), stop=(mt == MT - 1))
            # softmax denominator: row 64 of po[2] is sum(exp)
            rec = res.tile([1, QWIN], F32, tag="rec", name="rec", bufs=1)
            nc.vector.reciprocal(rec, po[2][64:65, :])
            pb = ps_mm.tile([128, QWIN], F32, tag="mm", name="mm")
            nc.tensor.matmul(pb, ONES32, rec, start=True, stop=True)
            dbc = res.tile([128, QWIN], F32, tag="dbc", name="dbc", bufs=1)
            nc.scalar.copy(out=dbc, in_=pb)
            at = []
            for cj, (off, sz, asz) in enumerate(CT):
                t = res.tile([asz, QWIN], BF16, tag=f"at_{cj}", name=f"at_{cj}", bufs=2)
                nc.vector.tensor_mul(t, po[cj], dbc[0:asz, :])
                at.append(t)
            for co, (off, sz, _) in enumerate(CT):
                pp = ps_mm.tile([sz, QWIN], F32, tag="mm", name="mm")
                for cj in range(3):
                    nc.tensor.matmul(
                        pp, WP[cj][:, off:off + sz], at[cj],
                        start=(cj == 0), stop=(cj == 2))
                resid_fn(co, qw, pp)

    # ======== attn1 (self-attention) ========
    kv_proj(X16, SAK, SAV)
    q_proj(XQ16, SAQ)

    def resid1(co, qw, pp):
        qsl = slice(qw * QWIN, (qw + 1) * QWIN)
        # x2 = 2*x + attn1
        nc.vector.scalar_tensor_tensor(
            out=X2[co][:, qsl], in0=XF[co][:, qsl], scalar=2.0, in1=pp,
            op0=mybir.AluOpType.mult, op1=mybir.AluOpType.add)

    attention_core(SAP, resid1)

    # ======== group-norm stats + AllReduce ========
    s12 = [res.tile([sz, 2], F32, tag=f"s12_{i}", name=f"s12_{i}", bufs=1) for i, (_, sz, _) in enumerate(CT)]
    scratch = res.tile([128, QL], F32, tag="scratch", name="scratch", bufs=1)
    for i, (_, sz, _) in enumerate(CT):
        nc.vector.reduce_sum(out=s12[i][:, 0:1], in_=X2[i], axis=mybir.AxisListType.X)
        nc.scalar.activation(
            out=scratch[0:sz, :], in_=X2[i],
            func=mybir.ActivationFunctionType.Square, accum_out=s12[i][:, 1:2])
    pg = ps_tiny.tile([GROUPS, 2], F32, tag="tiny", name="tiny")
    for i in range(3):
        nc.tensor.matmul(pg, INDsb[i], s12[i], start=(i == 0), stop=(i == 2))
    g12 = res.tile([GROUPS, 2], F32, tag="g12", name="g12", bufs=1)
    nc.vector.tensor_copy(out=g12, in_=pg)
    ccin = dram.tile([GROUPS, 2], F32, tag="ccin", name="ccin")
    ccout = dram.tile([GROUPS, 2], F32, tag="ccout", name="ccout")
    nc.sync.dma_start(out=ccin, in_=g12)
    nc.gpsimd.collective_compute(
        "AllReduce", mybir.AluOpType.add,
        replica_groups=[[0, 1, 2, 3], [4, 5, 6, 7]],
        ins=[ccin.opt()], outs=[ccout.opt()])
    gg = res.tile([GROUPS, 2], F32, tag="gg", name="gg", bufs=1)
    nc.sync.dma_start(out=gg, in_=ccout)

    # ======== attn2 K/V from context (independent of stats -> overlaps) ====
    C16 = []
    for i, (off, sz, asz) in enumerate(CT):
        t = rtile([asz, N], BF16, f"x16_{i}")  # reuse x16 slots
        nc.sync.dma_start(out=t, in_=ap["ct16"][off:off + asz, :])
        C16.append(t)
    kv_proj(C16, CAK, CAV)

    # ======== finish group norm ========
    gtmp = res.tile([GROUPS, 4], F32, tag="gtmp", name="gtmp", bufs=1)
    grp2 = res.tile([GROUPS, 2], F32, tag="grp2", name="grp2", bufs=1)
    inv = 1.0 / GCNT
    nc.vector.tensor_scalar_mul(out=grp2[:, 1:2], in0=gg[:, 0:1], scalar1=inv)   # mean
    nc.vector.tensor_scalar_mul(out=gtmp[:, 0:1], in0=gg[:, 1:2], scalar1=inv)   # E[x^2]
    nc.vector.tensor_mul(gtmp[:, 1:2], grp2[:, 1:2], grp2[:, 1:2])               # mean^2
    nc.vector.tensor_sub(gtmp[:, 2:3], gtmp[:, 0:1], gtmp[:, 1:2])               # var
    epst = res.tile([GROUPS, 1], F32, tag="epst", name="epst", bufs=1)
    nc.vector.memset(epst, float(EPS))
    nc.scalar.activation(out=gtmp[:, 3:4], in_=gtmp[:, 2:3],
                         func=mybir.ActivationFunctionType.Sqrt, bias=epst)
    nc.vector.reciprocal(grp2[:, 0:1], gtmp[:, 3:4])                             # rstd
    for i, (off, sz, _) in enumerate(CT):
        pc = ps_tiny.tile([sz, 2], F32, tag="tiny", name="tiny")
        nc.tensor.matmul(pc, INDT[:, off:off + sz], grp2, start=True, stop=True)
        scs = res.tile([sz, 4], F32, tag=f"scs_{i}", name=f"scs_{i}", bufs=1)
        nc.vector.tensor_mul(scs[:, 0:1], pc[:, 0:1], GB[i][:, 0:1])     # scale=rstd*gamma
        nc.vector.tensor_mul(scs[:, 3:4], pc[:, 1:2], scs[:, 0:1])      # mean*scale
        nc.vector.tensor_sub(scs[:, 1:2], GB[i][:, 1:2], scs[:, 3:4])   # shift
        nc.vector.tensor_scalar_add(out=scs[:, 2:3], in0=scs[:, 0:1], scalar1=1.0)
        # xn (bf16, for Q2 projection)
        nc.vector.tensor_scalar(
            out=XN16[i][0:sz, :], in0=X2[i], scalar1=scs[:, 0:1], scalar2=scs[:, 1:2],
            op0=mybir.AluOpType.mult, op1=mybir.AluOpType.add)
        # x2 <- x2 + xn  (= x2*(1+scale) + shift), fp32, in place
        nc.vector.tensor_scalar(
            out=X2[i], in0=X2[i], scalar1=scs[:, 2:3], scalar2=scs[:, 1:2],
            op0=mybir.AluOpType.mult, op1=mybir.AluOpType.add)

    # ======== attn2 ========
    q_proj(XN16, CAQ)

    def resid2(co, qw, pp):
        qsl = slice(qw * QWIN, (qw + 1) * QWIN)
        # x3 = (x2 + xn) + attn2
        nc.vector.tensor_add(X3[co][:, qsl], X2[co][:, qsl], pp)

    attention_core(CAP, resid2)
    for i, (_, sz, _) in enumerate(CT):
        nc.vector.tensor_copy(out=X316[i][0:sz, :], in_=X3[i])

    # ======== GEGLU FFN ========
    for qw in range(NQW):
        qsl = slice(qw * QWIN, (qw + 1) * QWIN)
        ff = []
        for hh in range(HT4):
            pa = ps_mm.tile([128, QWIN], F32, tag="mm", name="mm")
            pgg = ps_mm.tile([128, QWIN], F32, tag="mm", name="mm")
            for ci in range(3):
                nc.tensor.matmul(
                    pa, GW[ci][:, hh * 128:(hh + 1) * 128], X316[ci][:, qsl],
                    start=(ci == 0), stop=(ci == 2))
            for ci in range(3):
                nc.tensor.matmul(
                    pgg, GW[ci][:, C4 + hh * 128:C4 + (hh + 1) * 128], X316[ci][:, qsl],
                    start=(ci == 0), stop=(ci == 2))
            sg = res.tile([128, QWIN], F32, tag="sg", name="sg", bufs=2)
            nc.scalar.activation(out=sg, in_=pgg,
                                 func=mybir.ActivationFunctionType.Sigmoid, scale=1.702)
            gsg = res.tile([128, QWIN], BF16, tag="gsg", name="gsg", bufs=2)
            nc.vector.tensor_mul(gsg, pgg, sg)
            t = res.tile([128, QWIN], BF16, tag="ff", name="ff", bufs=HT4)
            nc.vector.tensor_mul(t, pa, gsg)
            ff.append(t)
        for co, (off, sz, _) in enumerate(CT):
            py = ps_mm.tile([sz, QWIN], F32, tag="mm", name="mm")
            for j in range(HT4):
                nc.tensor.matmul(py, DW[j][:, off:off + sz], ff[j],
                                 start=(j == 0), stop=False)
            nc.tensor.matmul(py, DB[:, off:off + sz], ONES16, start=False, stop=True)
            nc.vector.tensor_add(YT[co][:, qsl], py, X3[co][:, qsl])

    for i, (off, sz, _) in enumerate(CT):
        nc.sync.dma_start(out=ap["yt"][off:off + sz, :], in_=YT[i])


def _build(rep=1):
    key = rep
    if key in _NC_CACHE:
        return _NC_CACHE[key]
    nc = bacc.Bacc("TRN2", target_bir_lowering=False, debug=False, num_devices=NCORES)
    shapes = {
        "xt16": ([C + 1, N], BF16), "xq16": ([C + 1, QL], BF16),
        "ct16": ([C + 1, N], BF16), "xt32": ([C, QL], F32),
        "saq": ([C + 1, C], BF16), "sak": ([C + 1, C], BF16),
        "sav": ([C + 1, C], BF16), "sap": ([C + 1, C], BF16),
        "caq": ([C + 1, C], BF16), "cak": ([C + 1, C], BF16),
        "cav": ([C + 1, C], BF16), "cap": ([C + 1, C], BF16),
        "gw": ([C + 1, C8], BF16), "dw": ([C4, C], BF16), "db": ([1, C], BF16),
        "gb": ([C, 2], F32), "ind": ([C, GROUPS], F32), "indt": ([GROUPS, C], F32),
    }
    ap = {}
    for name, (shape, dt) in shapes.items():
        ap[name] = nc.dram_tensor(name, shape, dt, kind="ExternalInput").ap()
    ap["yt"] = nc.dram_tensor("yt", [C, QL], F32, kind="ExternalOutput").ap()

    with tile.TileContext(nc) as tc:
        with (
            tc.tile_pool(name="res", bufs=1) as res,
            tc.tile_pool(name="acc", bufs=3, space="PSUM") as acc,
            tc.tile_pool(name="mm", bufs=3, space="PSUM") as mm,
            tc.tile_pool(name="tiny", bufs=1, space="PSUM") as tiny,
            tc.tile_pool(name="dram", bufs=1, space="DRAM") as dram,
        ):
            pools = {"res": res, "acc": acc, "mm": mm, "tiny": tiny, "dram": dram}
            for _ in range(rep):
                _emit_body(nc, tc, ap, pools)
    nc.finalize()
    _NC_CACHE[key] = nc
    return nc


def _prep_inputs(inputs):
    """Host-side sharding/layout prep. Returns in_maps for the 8 cores."""
    f32 = np.float32

    def aug(w, b, scale=1.0):
        w = np.asarray(w, f32) * scale
        b = np.asarray(b, f32).reshape(1, -1) * scale
        return np.ascontiguousarray(np.concatenate([w, b], axis=0)).astype(bf16)

    x = np.asarray(inputs["x"], f32).reshape(B, N, C)
    ctx = np.asarray(inputs["context"], f32).reshape(B, N, C)
    xt = np.ascontiguousarray(x.transpose(0, 2, 1))      # [B, C, N] fp32
    ctxt = np.ascontiguousarray(ctx.transpose(0, 2, 1))

    ones_row = np.ones((1, N), f32)
    xt16 = [np.concatenate([xt[b], ones_row], axis=0).astype(bf16) for b in range(B)]
    ct16 = [np.concatenate([ctxt[b], ones_row], axis=0).astype(bf16) for b in range(B)]

    weights = {
        "saq": aug(inputs["sa_q_w"], inputs["sa_q_b"], SCALE),
        "sak": aug(inputs["sa_k_w"], inputs["sa_k_b"]),
        "sav": aug(inputs["sa_v_w"], inputs["sa_v_b"]),
        "sap": aug(inputs["sa_p_w"], inputs["sa_p_b"]),
        "caq": aug(inputs["ca_q_w"], inputs["ca_q_b"], SCALE),
        "cak": aug(inputs["ca_k_w"], inputs["ca_k_b"]),
        "cav": aug(inputs["ca_v_w"], inputs["ca_v_b"]),
        "cap": aug(inputs["ca_p_w"], inputs["ca_p_b"]),
        "gw": aug(inputs["geglu_w"], inputs["geglu_b"]),
        "dw": np.asarray(inputs["dense_w"], f32).astype(bf16),
        "db": np.asarray(inputs["dense_b"], f32).reshape(1, C).astype(bf16),
    }
    gb = np.stack([np.asarray(inputs["ca_norm_g"], f32),
                   np.asarray(inputs["ca_norm_b"], f32)], axis=1)  # [C, 2]
    ind = np.zeros((C, GROUPS), f32)
    ind[np.arange(C), np.arange(C) // GSIZE] = 1.0
    indt = np.ascontiguousarray(ind.T)

    in_maps = []
    for c in range(NCORES):
        b = c // 4
        q0 = (c % 4) * QL
        m = {
            "xt16": xt16[b],
            "xq16": np.ascontiguousarray(xt16[b][:, q0:q0 + QL]),
            "ct16": ct16[b],
            "xt32": np.ascontiguousarray(xt[b][:, q0:q0 + QL]),
            "gb": gb, "ind": ind, "indt": indt,
        }
        m.update(weights)
        in_maps.append(m)
    return in_maps


def kernel(**inputs):
    in_maps = _prep_inputs(inputs)
    nc = _build()
    res = run_bass_kernel_spmd(nc, in_maps, list(range(NCORES)))
    out = np.zeros((B, N, C), np.float32)
    for c in range(NCORES):
        b = c // 4
        q0 = (c % 4) * QL
        out[b, q0:q0 + QL, :] = res.results[c]["yt"].T
    return out.reshape(B, H, W, C)
